# revision 12
# baseline (speedup 1.0000x reference)
"""Trainium2 Bass kernel for nn_CrossAttention (self-attention, B=1 N=4096 D=640, 8 heads x 64).

Sharding: sequence-parallel over the 8 NeuronCores. Each core owns 512 query
rows; K/V are computed locally per-shard and all-gathered (bf16), attention
runs in a transposed layout (keys on partitions, queries on the free dim) so
no on-chip transposes are needed anywhere. Softmax denominators are fused
into the attn@v matmul via a ones-column appended to each head's V. The
output projection + bias happens on-chip; the host just concatenates the 8
row blocks. All matmuls run in bf16 with fp32 PSUM accumulation.

v2 notes:
- tiny dummy AllGather first to absorb the ~50us first-collective barrier
- scores/exp run LA chunks ahead of attn@v (software pipeline) so the V
  all-gather and ACT hiccups never block the PE stream
- small "filler" matmuls keep the PE busy so the HAM clock gate stays at
  2.4 GHz during the ACT-bound steady state
- normalization: denom broadcast by a K=1 matmul, fast-approx reciprocal,
  one DVE multiply per head; odd heads shifted to partitions 64-127 with a
  single SBUF->SBUF DMA so the output projection contracts K=128.
"""

import sys
import types

sys.path.insert(0, "/opt/trn_rl_repo")

import numpy as np
import ml_dtypes


# --- reconstruct the missing antenv.axon_hooks module (NTFF profiling) ------
def _ensure_axon_hooks():
    if "antenv.axon_hooks" in sys.modules:
        return
    holder = {"hook": None}
    mod = types.ModuleType("antenv.axon_hooks")
    mod.set_axon_ntff_profile_hook = lambda h: holder.__setitem__("hook", h)
    mod.get_axon_ntff_profile_hook = lambda: holder["hook"]
    sys.modules["antenv.axon_hooks"] = mod
    try:
        import antenv

        antenv.axon_hooks = mod
    except ImportError:
        pass
    try:
        from trn_agent_boot.trn_boot import _ntff_profile_via_ctypes

        mod.set_axon_ntff_profile_hook(
            _ntff_profile_via_ctypes("/opt/axon/libaxon_pjrt.so")
        )
    except Exception:
        pass


_ensure_axon_hooks()

import concourse.bass as bass
import concourse.mybir as mybir
import concourse.tile as tile
from concourse.tile import add_dep_helper
from concourse import bass_utils
from concourse.bass import ts
from concourse.bass_utils import run_bass_kernel_spmd

# fishfood upload is unavailable in this sandbox; trace path calls it
bass_utils.upload_artifacts = lambda tmpdir: "local://" + tmpdir

BF16 = mybir.dt.bfloat16
F32 = mybir.dt.float32
bf16 = ml_dtypes.bfloat16
EXP = mybir.ActivationFunctionType.Exp

R = 8  # cores / ranks
N = 4096  # sequence length
NL = N // R  # local sequence rows per core (512)
D = 640  # model dim
H = 8  # heads
DH = 64  # head dim
INNER = H * DH  # 512
KO = D // 128  # 5 contraction tiles for the projections
PAIRS = H // 2  # 4 head pairs
VW = DH + 1  # 65: v columns per head incl. the ones column
NCH = N // 128  # 32 key chunks
SCALE = DH**-0.5
LA = 27  # chunks of scores/exp lookahead ahead of attn@v


def _split_multi_waits(nc, max_waits=1):
    """walrus here rejects >1 wait per instruction; peel extras onto NoOps."""
    n = 0

    def fix(bb):
        nonlocal n
        out = []
        for ins in bb.instructions:
            blocks = getattr(ins, "blocks", None)
            if blocks:
                for b in blocks:
                    fix(b)
            si = getattr(ins, "sync_info", None)
            waits = list(si.on_wait) if (si is not None and si.on_wait) else []
            if len(waits) > max_waits:
                spill, keep = waits[:-max_waits], waits[-max_waits:]
                for w in spill:
                    out.append(
                        mybir.InstNoOp(
                            name=nc.get_next_instruction_name(),
                            engine=ins.engine,
                            sync_info=mybir.SyncInfo(on_wait=[w], on_update=[]),
                            bass_nofuse=True,
                        )
                    )
                ins.sync_info = mybir.SyncInfo(
                    on_wait=keep, on_update=list(si.on_update or [])
                )
                n += 1
            out.append(ins)
        bb.instructions = out

    for f in nc.m.functions:
        for bb in f.blocks:
            fix(bb)
    return n


def _build():
    nc = bass.Bass(num_devices=R)

    xT = nc.dram_tensor("xT", [D, NL], BF16, kind="ExternalInput")
    xTf = nc.dram_tensor("xTf", [D, N], BF16, kind="ExternalInput")
    wq = nc.dram_tensor("wq", [D, INNER], BF16, kind="ExternalInput")
    wk = nc.dram_tensor("wk", [D, INNER], BF16, kind="ExternalInput")
    wv = nc.dram_tensor("wv", [D, INNER], BF16, kind="ExternalInput")
    wo = nc.dram_tensor("wo", [INNER, D], BF16, kind="ExternalInput")
    bo = nc.dram_tensor("bo", [1, D], F32, kind="ExternalInput")
    out = nc.dram_tensor("out", [NL, D], F32, kind="ExternalOutput")

    v_cc_in, v_cc_out = [], []
    for half in range(2):
        v_cc_in.append(
            nc.dram_tensor(f"v_cc_in{half}", [NL, H * VW // 2], BF16, kind="Internal")
        )
        v_cc_out.append(
            nc.dram_tensor(
                f"v_cc_out{half}", [N, H * VW // 2], BF16, kind="Internal",
                addr_space="Shared",
            )
        )
    rgrp = [list(range(R))]

    with tile.TileContext(nc) as tc:
        with (
            tc.tile_pool(name="const", bufs=1) as cp,
            tc.tile_pool(name="work", bufs=3) as wp,
            tc.tile_pool(name="atp", bufs=LA + 3) as atp,
            tc.tile_pool(name="ps_kp", bufs=1, space="PSUM") as ps_kp,
            tc.tile_pool(name="big", bufs=2) as bp,
            tc.tile_pool(name="stage", bufs=2) as sp,
            tc.tile_pool(name="ps_sc", bufs=2, space="PSUM") as ps_sc,
            tc.tile_pool(name="ps_out", bufs=2, space="PSUM") as ps_out,
            tc.tile_pool(name="ps_mm", bufs=1, space="PSUM") as ps_mm,
        ):
            # ---- constants / weights in SBUF (chunked for DMA parallelism) -
            xt_sb = cp.tile([128, KO, NL], BF16, tag="xt")
            wk_sb = cp.tile([128, KO, INNER], BF16, tag="wk")
            wv_sb = cp.tile([128, KO, INNER], BF16, tag="wv")
            wq_sb = cp.tile([128, KO, INNER], BF16, tag="wq")
            xT_r = xT[:].rearrange("(ko p) s -> p ko s", p=128)
            wk_r = wk[:].rearrange("(ko p) m -> p ko m", p=128)
            wv_r = wv[:].rearrange("(ko p) m -> p ko m", p=128)
            wq_r = wq[:].rearrange("(ko p) m -> p ko m", p=128)
            for ko in range(KO):
                nc.sync.dma_start(xt_sb[:, ko : ko + 1, :], xT_r[:, ko : ko + 1, :])
                nc.sync.dma_start(wv_sb[:, ko : ko + 1, :], wv_r[:, ko : ko + 1, :])
            for ko in range(KO):
                nc.sync.dma_start(wq_sb[:, ko : ko + 1, :], wq_r[:, ko : ko + 1, :])
            for ko in range(KO):
                nc.sync.dma_start(wk_sb[:, ko : ko + 1, :], wk_r[:, ko : ko + 1, :])
            wo_sb = cp.tile([128, 4, D], BF16, tag="wo")
            wo_r = wo[:].rearrange("(t p) n -> p t n", p=128)
            for t in range(4):
                nc.sync.dma_start(wo_sb[:, t : t + 1, :], wo_r[:, t : t + 1, :])
            bo_sb = cp.tile([1, D], F32, tag="bo")
            nc.sync.dma_start(bo_sb[:], bo[:])
            ones_sb = cp.tile([1, 128], F32, tag="ones")
            nc.vector.memset(ones_sb[:], 1.0)
            ones_hi = cp.tile([65, 64], F32, tag="oneshi")
            nc.vector.memset(ones_hi[64:65, :], 1.0)

            qt_sb = cp.tile([128, 4, INNER], BF16, tag="qt")  # qT [inner, q]
            projT_sb = cp.tile([128, 4, NL], BF16, tag="projT")  # normalized outT
            odd_sb = cp.tile([64, 4, NL], BF16, tag="odd")  # odd heads pre-shift
            u_sb = cp.tile([64, H, NL], BF16, tag="u")  # unnormalized outT
            den_sb = cp.tile([65, 2, NL], F32, tag="den")  # denom staging @p64

            # warm the exp table early so the ~2.7us load overlaps phase 1
            warm = cp.tile([1, 8], F32, tag="warm")
            nc.scalar.activation(warm[0:1, 0:1], ones_sb[0:1, 0:1], EXP)

            # ---- phase 1 -----------------------------------------------------
            # v = x @ Wv, [seq, inner] padded to [seq, H*65] with ones columns
            # (first, so the V all-gather + its one-time barrier start ASAP)
            vt_sb = cp.tile([128, 4, H * VW], BF16, tag="vt")
            for so in range(4):
                pool = ps_mm if so == 2 else ps_sc
                shape = [128, NL] if pool is ps_mm else [128, 2 * NL]
                ps = pool.tile(shape, F32, tag="sc" if pool is ps_sc else "mm")
                for ko in range(KO):
                    nc.tensor.matmul(
                        ps[:, 0:INNER],
                        lhsT=xt_sb[:, ko, ts(so, 128)],
                        rhs=wv_sb[:, ko, :],
                        start=(ko == 0),
                        stop=(ko == KO - 1),
                    )
                dst = vt_sb[:, so, :].rearrange("p (h w) -> p h w", w=VW)
                nc.vector.tensor_copy(
                    dst[:, :, 0:DH], ps[:, 0:INNER].rearrange("p (h d) -> p h d", d=DH)
                )
                nc.vector.memset(dst[:, :, DH : DH + 1], 1.0)
            for half in range(2):
                nc.gpsimd.dma_start(
                    v_cc_in[half][:].rearrange("(so p) n -> p so n", p=128),
                    vt_sb[:, :, half * 4 * VW : (half + 1) * 4 * VW],
                )
                nc.gpsimd.collective_compute(
                    "AllGather", mybir.AluOpType.bypass, replica_groups=rgrp,
                    ins=[v_cc_in[half][:].opt()], outs=[v_cc_out[half][:].opt()],
                )

            # qT = (x @ Wq)^T, [inner, q] — stays in SBUF
            for mo in range(4):
                pool = ps_mm if mo == 2 else ps_sc
                shape = [128, NL] if pool is ps_mm else [128, 2 * NL]
                ps = pool.tile(shape, F32, tag="sc" if pool is ps_sc else "mm")
                for ko in range(KO):
                    nc.tensor.matmul(
                        ps[:, 0:NL],
                        lhsT=wq_sb[:, ko, ts(mo, 128)],
                        rhs=xt_sb[:, ko, :],
                        start=(ko == 0),
                        stop=(ko == KO - 1),
                    )
                nc.vector.tensor_copy(qt_sb[:, mo, :], ps[:, 0:NL])

            # full kT = Wk^T @ x_full^T computed LOCALLY on every core — this
            # replaces the K all-gather entirely. The 16 projection groups are
            # interleaved into the attention pipeline (emit_kproj_group) so
            # they fill the PE slack while the V all-gather is in flight.
            ktf_sb = cp.tile([128, 4, N], BF16, tag="ktf")
            xTf_r = xTf[:].rearrange("(ko p) s -> p ko s", p=128)

            def emit_kproj_group(g):
                # g 0..7 -> blocks 0,1 seq-chunk g; g 8..15 -> blocks 2,3
                sc_i, s8 = divmod(g, 8)
                xc = bp.tile([128, KO, NL], BF16, tag="xfc")
                for ko in range(KO):
                    nc.sync.dma_start(
                        xc[:, ko : ko + 1, :], xTf_r[:, ko : ko + 1, ts(s8, NL)]
                    )
                for idx, tb in enumerate((2 * sc_i, 2 * sc_i + 1)):
                    pool = ps_kp if idx == 0 else ps_mm
                    psk = pool.tile([128, NL], F32, tag="kp" if idx == 0 else "mm")
                    for ko in range(KO):
                        nc.tensor.matmul(
                            psk[:],
                            lhsT=wk_sb[:, ko, ts(tb, 128)],
                            rhs=xc[:, ko, :],
                            start=(ko == 0),
                            stop=(ko == KO - 1),
                        )
                    nc.vector.tensor_copy(ktf_sb[:, tb, ts(s8, NL)], psk[:])

            # gathered V access patterns (per half)
            vg = [
                v_cc_out[half][:].rearrange("(g p) n -> p g n", p=128)
                for half in range(2)
            ]

            # ---- phase 2: attention, globally software-pipelined ------------
            # one stream of 128 chunk-units (4 pairs x 32 chunks); scores/exp
            # run LA units ahead of attn@v so ACT never idles, even across
            # pair boundaries.
            TOT = PAIRS * NCH
            vb_bigs, outps, at_tiles = {}, {}, {}

            def emit_pair_prefetch(t):
                half, tl = t // 2, t % 2
                vb_big = bp.tile([128, NCH, 2 * VW], BF16, tag="vbbig")
                for g in range(0, NCH, 4):
                    nc.gpsimd.dma_start(
                        vb_big[:, g : g + 4, :],
                        vg[half][:, g : g + 4, tl * 2 * VW : (tl + 1) * 2 * VW],
                    )
                vb_bigs[t] = vb_big

            rb_all = cp.tile([64, H, NL], F32, tag="rball")

            def emit_pair_norm(t):
                # only cheap PE/DVE work here; ACT Ln/Exp batched per half
                outp1, outp2 = outps.pop(t)
                for h01 in range(2):
                    h = 2 * t + h01
                    outp = outp1 if h01 == 0 else outp2
                    nc.vector.tensor_copy(u_sb[:, h, :], outp[0:64, :])
                    nc.vector.tensor_copy(den_sb[64:65, h01, :], outp[64:65, :])
                for h01 in range(2):
                    h = 2 * t + h01
                    bc = ps_mm.tile([64, NL], F32, tag="mm")
                    nc.tensor.matmul(
                        bc[:], lhsT=ones_hi[64:65, :], rhs=den_sb[64:65, h01, :],
                        start=True, stop=True,
                    )
                    nc.vector.tensor_copy(rb_all[:, h, :], bc[:])
                if t % 2 == 1:
                    # finish normalization for this half: recip = exp(-ln(den))
                    hf = t // 2
                    sl = rb_all[:, 4 * hf : 4 * hf + 4, :]
                    nc.scalar.activation(sl, sl, mybir.ActivationFunctionType.Ln)
                    nc.scalar.activation(sl, sl, EXP, scale=-1.0)
                    for tt in (2 * hf, 2 * hf + 1):
                        for h01 in range(2):
                            h = 2 * tt + h01
                            if h01 == 0:
                                nc.vector.tensor_mul(
                                    out=projT_sb[0:64, tt, :], in0=u_sb[:, h, :],
                                    in1=rb_all[:, h, :],
                                )
                            else:
                                nc.vector.tensor_mul(
                                    out=odd_sb[:, tt, :], in0=u_sb[:, h, :],
                                    in1=rb_all[:, h, :],
                                )

            def emit_attnv(j, anchor):
                tj, cj = divmod(j, NCH)
                at = at_tiles.pop(j)
                outp1, outp2 = outps[tj]
                vb_big = vb_bigs[tj]
                mm1 = nc.tensor.matmul(
                    outp1[0:VW, :],
                    lhsT=vb_big[:, cj, 0:VW],
                    rhs=at[:, 0:NL],
                    start=(cj == 0), stop=(cj == NCH - 1),
                )
                if anchor is not None:
                    # keep attn@v behind the lookahead scores in the PE stream;
                    # the Tile scheduler would otherwise hoist it to right
                    # after exp(j), where a pending V wait stalls the PE FIFO
                    add_dep_helper(
                        mm1.ins, anchor, sync=False, reason="attnv after lookahead"
                    )
                nc.tensor.matmul(
                    outp2[0:VW, :],
                    lhsT=vb_big[:, cj, VW : 2 * VW],
                    rhs=at[:, NL : 2 * NL],
                    start=(cj == 0), stop=(cj == NCH - 1),
                )
                if cj == NCH - 1:
                    emit_pair_norm(tj)

            # kproj group g must land before the unit that first reads it:
            # groups 0..7 feed units 4*s8 (pairs 0,1); 8..15 feed 64+4*s8.
            kproj_sched = {}
            for g in range(2):
                kproj_sched.setdefault(0, []).append(g)  # prologue
            for g in range(2, 8):
                kproj_sched.setdefault(max(0, 4 * g - 10), []).append(g)
            # groups for pairs 2,3 run during the V-allgather stall window
            for g in range(8, 16):
                kproj_sched.setdefault(22 + (g - 8), []).append(g)

            attnv_next = 0
            last_score = None
            for i in range(TOT):
                t, c = divmod(i, NCH)
                for g in kproj_sched.get(i, ()):
                    emit_kproj_group(g)
                if c == 0:
                    emit_pair_prefetch(t)
                    op1 = ps_out.tile([128, NL], F32, tag="outp", name="op1")
                    op2 = ps_out.tile([128, NL], F32, tag="outp", name="op2")
                    outps[t] = (op1, op2)
                sc = ps_sc.tile([128, 2 * NL], F32, tag="sc")
                nc.tensor.matmul(
                    sc[:, 0:NL],
                    lhsT=ktf_sb[0:64, t, ts(c, 128)],
                    rhs=qt_sb[0:64, t, :],
                    start=True, stop=True,
                )
                s2 = nc.tensor.matmul(
                    sc[:, NL : 2 * NL],
                    lhsT=ktf_sb[64:128, t, ts(c, 128)],
                    rhs=qt_sb[64:128, t, :],
                    start=True, stop=True,
                )
                last_score = s2.ins
                at = atp.tile([128, 2 * NL], BF16, tag="at")
                nc.scalar.activation(at[:], sc[:], EXP, scale=SCALE)
                at_tiles[i] = at
                # full lookahead only while attn@v is gated on the V
                # all-gather (pairs 0,1); then shrink it so the tail is short
                lag = LA if attnv_next < 2 * NCH else 6
                while attnv_next <= i - lag:
                    emit_attnv(attnv_next, last_score)
                    attnv_next += 1
            while attnv_next < TOT:
                emit_attnv(attnv_next, last_score)
                attnv_next += 1

            # shift odd heads to partitions 64-127 (one SBUF->SBUF DMA)
            nc.sync.dma_start(projT_sb[64:128, :, :], odd_sb[:])

            # ---- phase 3: output projection + bias -------------------------
            for n0, nw in ((0, 512), (512, 128)):
                for so in range(4):
                    pool = ps_mm if so % 2 == 0 else ps_sc
                    shape = [128, NL] if pool is ps_mm else [128, 2 * NL]
                    f = pool.tile(shape, F32, tag="sc" if pool is ps_sc else "mm")
                    for t in range(4):
                        nc.tensor.matmul(
                            f[:, 0:nw],
                            lhsT=projT_sb[:, t, ts(so, 128)],
                            rhs=wo_sb[:, t, n0 : n0 + nw],
                            start=(t == 0), stop=False,
                        )
                    nc.tensor.matmul(
                        f[:, 0:nw],
                        lhsT=ones_sb[0:1, :],
                        rhs=bo_sb[0:1, n0 : n0 + nw],
                        start=False, stop=True,
                    )
                    o_sb = sp.tile([128, 512], F32, tag="osb")
                    nc.scalar.copy(o_sb[:, 0:nw], f[:, 0:nw])
                    nc.sync.dma_start(out[ts(so, 128), n0 : n0 + nw], o_sb[:, 0:nw])

    _split_multi_waits(nc)
    return nc


_NC_CACHE = {}


def _get_nc():
    if "nc" not in _NC_CACHE:
        _NC_CACHE["nc"] = _build()
    return _NC_CACHE["nc"]


def _prep_inputs(x, Wq, Wk, Wv, Wo, bo):
    x2 = np.asarray(x, dtype=np.float32).reshape(N, D)
    wq_b = np.asarray(Wq, dtype=np.float32).astype(bf16)
    wk_b = np.asarray(Wk, dtype=np.float32).astype(bf16)
    wv_b = np.asarray(Wv, dtype=np.float32).astype(bf16)
    wo_b = np.asarray(Wo, dtype=np.float32).astype(bf16)
    bo_f = np.asarray(bo, dtype=np.float32).reshape(1, D)
    xTf = np.ascontiguousarray(x2.T).astype(bf16)
    in_maps = []
    for r in range(R):
        xT = np.ascontiguousarray(x2[r * NL : (r + 1) * NL, :].T).astype(bf16)
        in_maps.append(
            {
                "xT": xT, "xTf": xTf, "wq": wq_b, "wk": wk_b, "wv": wv_b,
                "wo": wo_b, "bo": bo_f,
            }
        )
    return in_maps


def run(x, Wq, Wk, Wv, Wo, bo, trace=False):
    nc = _get_nc()
    in_maps = _prep_inputs(x, Wq, Wk, Wv, Wo, bo)
    res = run_bass_kernel_spmd(nc, in_maps, core_ids=list(range(R)), trace=trace)
    full = np.concatenate([res.results[r]["out"] for r in range(R)], axis=0)
    return full.reshape(1, N, D), res


def kernel(x, Wq, Wk, Wv, Wo, bo):
    out, _ = run(x, Wq, Wk, Wv, Wo, bo, trace=False)
    return out
